# revision 35
# baseline (speedup 1.0000x reference)
"""Multi-head causal attention block on 8 Trainium2 NeuronCores.

Problem: x[8,1024,1024] @ W_qkv[1024,3072] -> causal MHA (16 heads x 64) ->
@ W_out[1024,1024].  Data-parallel: one batch element per core; weights
replicated.

Layout strategy (per core, everything transposed so no intermediate
transposes are needed):
  xT  [128, di, s]        via PE transposes (f32r, 1.5 cyc/row)
  qT,kT = (x W_{q,k})^T   [2D, S] bf16   lhsT=W_qkv tiles, rhs=xT
  v'' [s-tile, h, 65] bf16 with a ones column per head (softmax sums)
  scores^T = k_h q_h^T    [S, S] per head, causal-trimmed, K=64, bf16
  P^T = exp(scores^T/8)   bf16, no max-subtraction (|s|/8 <= ~3.2)
  O'^T = v''_h^T P^T      [65, S]  row 64 = softmax denominators
  oT  = O^T / sums        bf16, DVE mult by Pool-broadcast 1/sums
  out = (oT)^T W_out      W_out converted to bf16 on load

Engine budget (sim): PE ~180us; Act = exp(~97us) + xT/v/out copies placed
in Act-idle phases; DVE = qkT copies + recip + normalize; Pool = causal
affine_selects (post-exp, SBUF-only: GPSIMD has no PSUM port) + bcasts.
QKV projection is interleaved with attention per head-pair so exp overlaps
projection matmuls instead of serializing after them.
"""
import sys
sys.path.insert(0, "/opt/trn_rl_repo")
from contextlib import ExitStack

import numpy as np

import concourse.bass as bass
import concourse.bacc as bacc
import concourse.mybir as mybir
import concourse.tile as tile
from concourse.bass_utils import run_bass_kernel_spmd
from concourse.masks import make_identity

F32 = mybir.dt.float32
F32R = mybir.dt.float32r
BF16 = mybir.dt.bfloat16
ACT_COPY = mybir.ActivationFunctionType.Copy
ACT_EXP = mybir.ActivationFunctionType.Exp

B, S, D, H, DH = 8, 1024, 1024, 16, 64
NCORES = 8
ST = S // 128          # 8 seq tiles
DT = D // 128          # 8 feature tiles
QB = S // 512          # 2 query blocks of 512


def build_nc(repeat: int = 1, cut: str = "", ablate: str = "") -> "bacc.Bacc":
    nc = bacc.Bacc()
    x_d = nc.dram_tensor("x", [S, D], F32, kind="ExternalInput")
    wqkv_d = nc.dram_tensor("wqkv", [D, 3 * D], F32R, kind="ExternalInput")
    wout_d = nc.dram_tensor("wout", [D, D], F32R, kind="ExternalInput")
    out_d = nc.dram_tensor("out", [S, D], F32, kind="ExternalOutput")

    wqkv_v = wqkv_d[:, :].rearrange("(dt p) e -> p dt e", p=128)
    x_v = x_d[:, :].rearrange("(g p) d -> p g d", p=128)

    with tile.TileContext(nc) as tc, ExitStack() as top:
        pool = lambda *a, **k: top.enter_context(tc.tile_pool(*a, **k))  # noqa
        consts = pool(name="consts", bufs=1)
        xtp = pool(name="xtp", bufs=1)     # xT big [128, 8, 1024] f32r
        qkp = pool(name="qkp", bufs=1)     # qkT 16x [128,1024] bf16
        vp = pool(name="vp", bufs=1)       # vpp 8x [128,16,65] bf16
        otp = pool(name="otp", bufs=1)     # oT 8x [128,1024] bf16
        wop = pool(name="wop", bufs=1)     # wout bf16 8x [128,1024]
        xsp = pool(name="xsp", bufs=2)     # x staging [128,1024] f32r
        wqs = pool(name="wqs", bufs=2)     # wqr staging [128,8,128]
        wvp = pool(name="wvp", bufs=1)     # wv [128,8,1024] f32r
        wos = pool(name="wos", bufs=2)     # wout f32r staging [128,1024]
        ptp = pool(name="ptp", bufs=8)     # pt [128,512] bf16
        rsp = pool(name="rsp", bufs=3)     # rs [1,512] f32
        bcp = pool(name="bcp", bufs=3)     # bc [64,512] f32
        osp = pool(name="osp", bufs=2)     # out staging [128,512] f32
        psA = pool(name="psA", bufs=5, space="PSUM")   # [128,512] accums
        psO = pool(name="psO", bufs=3, space="PSUM")   # [65,512] PV accums

        ident = consts.tile([128, 128], F32, tag="ident", name="ident")
        make_identity(nc, ident)

        for rep in range(repeat):
            r = f"r{rep}"
            xT = xtp.tile([128, DT, S], F32R, tag="xT", name=f"xT{r}")
            qkT = [qkp.tile([128, S], BF16, tag=f"qkT{fi}",
                            name=f"qkT{fi}{r}") for fi in range(2 * DT)]
            vpp = [vp.tile([128, H, DH + 1], BF16, tag=f"vpp{si}",
                           name=f"vpp{si}{r}") for si in range(ST)]
            oT = [otp.tile([128, S], BF16, tag=f"oT{fi}",
                           name=f"oT{fi}{r}") for fi in range(DT)]
            wv = wvp.tile([128, DT, S], F32R, tag="wv", name=f"wv{r}")
            wor = [wop.tile([128, D], BF16, tag=f"wor{fi}",
                            name=f"wor{fi}{r}") for fi in range(DT)]

            # ---- stage 0: load x chunks, transpose on PE, copy out on Act
            def emit_T(g, split=False):
                xs = xsp.tile([128, D], F32, tag="xs", name=f"xs{g}{r}")
                if split:
                    # halves in separate DMAs so the first transposes start
                    # ~0.7us earlier (startup is DMA-latency-bound)
                    nc.sync.dma_start(out=xs[:, 0:512], in_=x_v[:, g, 0:512])
                    nc.sync.dma_start(out=xs[:, 512:1024],
                                      in_=x_v[:, g, 512:1024])
                else:
                    nc.sync.dma_start(out=xs, in_=x_v[:, g, :])
                for half in range(2):
                    tp = psA.tile([128, 512], F32, tag="ps",
                                  name=f"tp{g}_{half}{r}")
                    for q in range(4):
                        di = half * 4 + q
                        nc.tensor.transpose(
                            tp[:, q * 128:(q + 1) * 128],
                            xs[:, di * 128:(di + 1) * 128], ident)
                    nc.scalar.activation(
                        xT[:, half * 4:(half + 1) * 4, g * 128:(g + 1) * 128],
                        tp.rearrange("p (q c) -> p q c", q=4), ACT_COPY)

            # ones columns for softmax denominators (Pool, SBUF-only)
            def emit_ones():
                for si in range(ST):
                    nc.gpsimd.memset(vpp[si][:, :, DH:DH + 1], 1.0)

            # ---- weight DMAs (queue order = need order) ----
            def load_wqr(fi):
                w = wqs.tile([128, DT, 128], F32R, tag="wqr",
                             name=f"wqr{fi}{r}")
                nc.sync.dma_start(out=w, in_=wqkv_v[:, :, fi * 128:(fi + 1) * 128])
                return w

            def gen_qkT(fi, w, cbs=(0, 1)):
                """Generator: yields pe_ns after every ~2 matmuls so attention
                emission can interleave (PE's in-order queue only skips 4
                blocked instructions, so long blocked PV runs stall it)."""
                for cb in cbs:
                    ps = psA.tile([128, 512], F32, tag="ps",
                                  name=f"qkacc{fi}_{cb}{r}")
                    for di in range(DT):
                        nc.tensor.matmul(
                            ps, w[:, di, :],
                            xT[:, di, cb * 512:(cb + 1) * 512],
                            start=(di == 0), stop=(di == DT - 1))
                        if di % 2 == 1:
                            yield 426
                    nc.vector.tensor_copy(
                        qkT[fi][:, cb * 512:(cb + 1) * 512], ps)
                    yield 0

            def emit_qkT(fi, w, cbs=(0, 1)):
                for _ in gen_qkT(fi, w, cbs):
                    pass

            def gen_v(vb, copy_engine):
                # v natural [s, f] for heads vb*8..vb*8+7 -> vpp bf16
                for si in range(ST):
                    ps = psA.tile([128, 512], F32, tag="ps",
                                  name=f"vacc{vb}_{si}{r}")
                    for di in range(DT):
                        nc.tensor.matmul(
                            ps, xT[:, di, si * 128:(si + 1) * 128],
                            wv[:, di, vb * 512:(vb + 1) * 512],
                            start=(di == 0), stop=(di == DT - 1))
                        if di % 2 == 1:
                            yield 426
                    copy_engine(
                        vpp[si][:, vb * 8:(vb + 1) * 8, 0:DH],
                        ps.rearrange("p (h d) -> p h d", h=8))
                    yield 0

            def emit_v(vb, copy_engine):
                for _ in gen_v(vb, copy_engine):
                    pass

            def act_copy(dst, src):
                nc.scalar.activation(dst, src, ACT_COPY)

            def dve_copy(dst, src):
                nc.vector.tensor_copy(dst, src)

            PIPE = 2

            # filler machinery: a FIFO of (label, generator) yielding pe_ns
            # chunks; attention emission drains fillers to cover its
            # Act-vs-PE deficit.  drain_until forces a dependency's filler
            # to be fully emitted before its consumer.
            fillers: list = []
            deficit = [0.0]

            def drain(ns):
                while ns > 0 and fillers:
                    try:
                        ns -= next(fillers[0][1])
                    except StopIteration:
                        fillers.pop(0)
                return ns

            def drain_until(label):
                while any(lb == label for lb, _ in fillers):
                    lb, g = fillers[0]
                    for _ in g:
                        pass
                    fillers.pop(0)

            def drain_all():
                while fillers:
                    drain(1e9)

            def emit_attn(qb, hp):
                drain_until(("qkT", hp))
                if hp >= 4:
                    drain_until(("v", 1))
                kmax = 4 * (qb + 1)
                fq, fk = hp, DT + hp
                o_ps = [psO.tile([DH + 1, 512], F32, tag="ops",
                                 name=f"o{qb}_{hp}_{j}{r}") for j in range(2)]
                pts: dict = {}

                def emit_scores(ki):
                    qs = max(0, ki * 128 - qb * 512)
                    n = 512 - qs
                    diag = ki * 128 >= qb * 512
                    for j in range(2):
                        p0 = j * 64
                        s_ps = psA.tile([128, 512], F32, tag="ps",
                                        name=f"s{qb}_{hp}_{ki}_{j}{r}")
                        nc.tensor.matmul(
                            s_ps[:, 0:n],
                            qkT[fk][p0:p0 + 64, ki * 128:(ki + 1) * 128],
                            qkT[fq][p0:p0 + 64, qb * 512 + qs:(qb + 1) * 512],
                            start=True, stop=True)
                        pt = ptp.tile([128, 512], BF16, tag="pt",
                                      name=f"pt{qb}_{hp}_{ki}_{j}{r}")
                        nc.scalar.activation(
                            pt[:, 0:n], s_ps[:, 0:n], ACT_EXP, scale=0.125)
                        if diag:
                            # causal: zero keys below the diagonal (Pool,
                            # post-exp, SBUF) -- keep where query >= key
                            nc.gpsimd.affine_select(
                                out=pt[:, 0:128], in_=pt[:, 0:128],
                                compare_op=mybir.AluOpType.is_ge,
                                fill=0.0, base=0, pattern=[[1, 128]],
                                channel_multiplier=-1)
                        pts[(ki, j)] = (pt, qs, n)

                def emit_pv(ki):
                    for j in range(2):
                        pt, qs, n = pts.pop((ki, j))
                        nc.tensor.matmul(
                            o_ps[j][:, qs:512],
                            vpp[ki][:, 2 * hp + j, :], pt[:, 0:n],
                            start=(ki == 0), stop=(ki == kmax - 1))

                for ki in range(kmax):
                    emit_scores(ki)
                    pe = act = 0.0
                    n = 512 - max(0, ki * 128 - qb * 512)
                    pe += 2 * n * 0.4166
                    act += 2 * (n * 0.833 + 190)
                    if ki >= PIPE:
                        emit_pv(ki - PIPE)
                        pe += 2 * (512 - max(0, (ki - PIPE) * 128
                                             - qb * 512)) * 0.4166
                    deficit[0] += act - pe
                    if deficit[0] > 0:
                        deficit[0] = drain(deficit[0])
                for ki in range(max(0, kmax - PIPE), kmax):
                    emit_pv(ki)

                for j in range(2):
                    p0 = j * 64
                    rs = rsp.tile([1, 512], F32, tag="rs",
                                  name=f"rs{qb}_{hp}_{j}{r}")
                    nc.vector.reciprocal(rs, o_ps[j][DH:DH + 1, :])
                    bc = bcp.tile([64, 512], F32, tag="bc",
                                  name=f"bc{qb}_{hp}_{j}{r}")
                    nc.gpsimd.partition_broadcast(bc, rs)
                    nc.vector.tensor_mul(
                        oT[hp][p0:p0 + 64, qb * 512:(qb + 1) * 512],
                        o_ps[j][0:DH, :], bc)

            def gen_out_proj(qb, copy_engine):
                for si in range(4 * qb, 4 * (qb + 1)):
                    for eb in range(2):
                        ps = psA.tile([128, 512], F32, tag="ps",
                                      name=f"op{si}_{eb}{r}")
                        for fi in range(DT):
                            nc.tensor.matmul(
                                ps, oT[fi][:, si * 128:(si + 1) * 128],
                                wor[fi][:, eb * 512:(eb + 1) * 512],
                                start=(fi == 0), stop=(fi == DT - 1))
                            if fi % 2 == 1:
                                yield 426
                        ostg = osp.tile([128, 512], F32, tag="ostg",
                                        name=f"ostg{si}_{eb}{r}")
                        copy_engine(ostg, ps)
                        nc.sync.dma_start(
                            out=out_d[si * 128:(si + 1) * 128,
                                      eb * 512:(eb + 1) * 512],
                            in_=ostg)
                        yield 0

            def out_proj(qb, copy_engine):
                for _ in gen_out_proj(qb, copy_engine):
                    pass

            # ---- schedule ----
            # Explicit interleave: attention units (qb,hp) spread across the
            # projection stream so Act's exp load (97us) never locally
            # outruns PE.  q1 units are Act-heavy (8.7us exp vs 3.7 for q0),
            # so they sit next to big PE segments.
            def load_wv(vb):
                for di in range(DT):
                    nc.sync.dma_start(
                        out=wv[:, di, vb * 512:(vb + 1) * 512],
                        in_=wqkv_d[di * 128:(di + 1) * 128,
                                   2 * D + vb * 512:2 * D + (vb + 1) * 512])

            def gen_wout(k):
                # wout via ordered sync queue (a dep-free gpsimd casting DMA
                # gets scheduled at t=0 and clogs startup DMA bandwidth);
                # f32->bf16 convert on DVE
                for fi in (2 * k, 2 * k + 1):
                    ws = wos.tile([128, D], F32R, tag="wos",
                                  name=f"wos{fi}{r}")
                    nc.sync.dma_start(
                        out=ws, in_=wout_d[fi * 128:(fi + 1) * 128, :])
                    nc.vector.tensor_copy(wor[fi], ws)
                    yield 0

            def filler_qkT(hp):
                # DMA now (queue position), matmuls drained later
                wq_ = load_wqr(hp)
                wk_ = load_wqr(hp + DT)
                fillers.append((("qkT", hp), gen_qkT(hp, wq_)))
                fillers.append((("qkT", hp), gen_qkT(hp + DT, wk_)))

            # startup: T(g0-3) -> qkT(h0) cb0 halves -> T(g4-7) -> rest,
            # DMA queue ordered to match (x0-3, wqr h0, x4-7, wv0, wqr h1,
            # wv1, then per-hp wqr / wout)
            for g in range(4):
                emit_T(g, split=(g == 0))
            w00 = load_wqr(0)
            w01 = load_wqr(DT)
            emit_qkT(0, w00, cbs=(0,))
            emit_qkT(DT, w01, cbs=(0,))
            for g in range(4, ST):
                emit_T(g)
            emit_ones()
            load_wv(0)
            emit_qkT(0, w00, cbs=(1,))
            emit_qkT(DT, w01, cbs=(1,))
            w10 = load_wqr(1)
            w11 = load_wqr(1 + DT)
            load_wv(1)
            emit_qkT(1, w10)
            emit_qkT(1 + DT, w11)
            emit_v(0, act_copy)
            # attention with projection fillers interleaved at matmul
            # granularity; filler order respects needs (qkT h+1 completes
            # during units of h-1)
            filler_qkT(2)
            fillers.append(("wout", gen_wout(0)))
            emit_attn(0, 0)
            emit_attn(1, 0)
            filler_qkT(3)
            fillers.append(("wout", gen_wout(1)))
            emit_attn(0, 1)
            emit_attn(1, 1)
            filler_qkT(4)
            fillers.append(("wout", gen_wout(2)))
            emit_attn(0, 2)
            emit_attn(1, 2)
            fillers.append((("v", 1), gen_v(1, dve_copy)))
            filler_qkT(5)
            fillers.append(("wout", gen_wout(3)))
            emit_attn(0, 3)
            emit_attn(1, 3)
            filler_qkT(6)
            emit_attn(0, 4)
            emit_attn(1, 4)
            filler_qkT(7)
            emit_attn(0, 5)
            emit_attn(1, 5)
            emit_attn(0, 6)
            emit_attn(1, 6)
            drain_all()
            emit_attn(0, 7)
            fillers.append(("oproj", gen_out_proj(0, dve_copy)))
            emit_attn(1, 7)
            drain_all()
            out_proj(1, act_copy)
    nc.compile()
    return nc


_nc_cache: dict = {}


def _get_nc(repeat: int = 1, cut: str = ""):
    if (repeat, cut) not in _nc_cache:
        _nc_cache[(repeat, cut)] = build_nc(repeat, cut)
    return _nc_cache[(repeat, cut)]


def run(x, W_qkv, W_out, repeat: int = 1):
    nc = _get_nc(repeat)
    x = np.ascontiguousarray(np.asarray(x, dtype=np.float32))
    W_qkv = np.ascontiguousarray(np.asarray(W_qkv, dtype=np.float32))
    W_out = np.ascontiguousarray(np.asarray(W_out, dtype=np.float32))
    in_maps = [{"x": x[b], "wqkv": W_qkv, "wout": W_out} for b in range(NCORES)]
    res = run_bass_kernel_spmd(nc, in_maps, core_ids=list(range(NCORES)))
    return np.stack([res.results[b]["out"] for b in range(NCORES)], axis=0)


def kernel(x, mask=None, W_qkv=None, W_out=None):
    """Full-input entry point; mask is always causal-tril and is hardcoded."""
    return run(x, W_qkv, W_out, repeat=1)


# revision 40
# speedup vs baseline: 1.1656x; 1.1656x over previous
"""Multi-head causal attention block on 8 Trainium2 NeuronCores.

Problem: x[8,1024,1024] @ W_qkv[1024,3072] -> causal MHA (16 heads x 64) ->
@ W_out[1024,1024].  Data-parallel: one batch element per core; weights
replicated.

Layout strategy (per core, everything transposed so no intermediate
transposes are needed):
  xT  [128, di, s]        via PE transposes (f32r, 1.5 cyc/row)
  qT,kT = (x W_{q,k})^T   [2D, S] bf16   lhsT=W_qkv tiles, rhs=xT
  v'' [s-tile, h, 65] bf16 with a ones column per head (softmax sums)
  scores^T = k_h q_h^T    [S, S] per head, causal-trimmed, K=64, bf16
  P^T = exp(scores^T/8)   bf16, no max-subtraction (|s|/8 <= ~3.2)
  O'^T = v''_h^T P^T      [65, S]  row 64 = softmax denominators
  oT  = O^T / sums        bf16, DVE mult by Pool-broadcast 1/sums
  out = (oT)^T W_out      W_out converted to bf16 on load

Engine budget (sim): PE ~180us; Act = exp(~97us) + xT/v/out copies placed
in Act-idle phases; DVE = qkT copies + recip + normalize; Pool = causal
affine_selects (post-exp, SBUF-only: GPSIMD has no PSUM port) + bcasts.
QKV projection is interleaved with attention per head-pair so exp overlaps
projection matmuls instead of serializing after them.
"""
import sys
sys.path.insert(0, "/opt/trn_rl_repo")
from contextlib import ExitStack

import numpy as np

import concourse.bass as bass
import concourse.bacc as bacc
import concourse.mybir as mybir
import concourse.tile as tile
from concourse.bass_utils import run_bass_kernel_spmd
from concourse.masks import make_identity

F32 = mybir.dt.float32
F32R = mybir.dt.float32r
BF16 = mybir.dt.bfloat16
ACT_COPY = mybir.ActivationFunctionType.Copy
ACT_EXP = mybir.ActivationFunctionType.Exp

B, S, D, H, DH = 8, 1024, 1024, 16, 64
NCORES = 8
ST = S // 128          # 8 seq tiles
DT = D // 128          # 8 feature tiles
QB = S // 512          # 2 query blocks of 512


def build_nc(repeat: int = 1, cut: str = "", ablate: str = "") -> "bacc.Bacc":
    nc = bacc.Bacc()
    x_d = nc.dram_tensor("x", [S, D], F32, kind="ExternalInput")
    wqkv_d = nc.dram_tensor("wqkv", [D, 3 * D], F32R, kind="ExternalInput")
    wout_d = nc.dram_tensor("wout", [D, D], F32R, kind="ExternalInput")
    out_d = nc.dram_tensor("out", [S, D], F32, kind="ExternalOutput")

    wqkv_v = wqkv_d[:, :].rearrange("(dt p) e -> p dt e", p=128)
    x_v = x_d[:, :].rearrange("(g p) d -> p g d", p=128)

    with tile.TileContext(nc) as tc, ExitStack() as top:
        pool = lambda *a, **k: top.enter_context(tc.tile_pool(*a, **k))  # noqa
        consts = pool(name="consts", bufs=1)
        xtp = pool(name="xtp", bufs=1)     # xT big [128, 8, 1024] f32r
        qkp = pool(name="qkp", bufs=1)     # qkT 16x [128,1024] bf16
        vp = pool(name="vp", bufs=1)       # vpp 8x [128,16,65] bf16
        otp = pool(name="otp", bufs=1)     # oT 8x [128,1024] bf16
        wop = pool(name="wop", bufs=1)     # wout bf16 8x [128,1024]
        xsp = pool(name="xsp", bufs=2)     # x staging [128,1024] f32r
        wqs = pool(name="wqs", bufs=2)     # wqr staging [128,8,128]
        wvp = pool(name="wvp", bufs=1)     # wv [128,8,1024] f32r
        wos = pool(name="wos", bufs=2)     # wout f32r staging [128,1024]
        ptp = pool(name="ptp", bufs=8)     # pt [128,512] bf16
        obp = pool(name="obp", bufs=4)     # osb [65,512] bf16
        rsp = pool(name="rsp", bufs=3)     # rs [1,512] bf16
        bcp = pool(name="bcp", bufs=3)     # bc [64,512] bf16
        osp = pool(name="osp", bufs=2)     # out staging [128,512] f32
        psA = pool(name="psA", bufs=5, space="PSUM")   # [128,512] accums
        psO = pool(name="psO", bufs=3, space="PSUM")   # [65,512] PV accums

        ident = consts.tile([128, 128], F32, tag="ident", name="ident")
        make_identity(nc, ident)
        # bf16 causal keep-mask for diag tiles: 1 where query(free) >=
        # key(channel), 0 below -- applied as a DVE multiply (2x bf16 mode),
        # keeping Pool's in-order queue out of the exp->PV chain
        trimask = consts.tile([128, 128], BF16, tag="trimask", name="trimask")
        nc.gpsimd.memset(trimask, 1.0)
        nc.gpsimd.affine_select(
            out=trimask, in_=trimask, compare_op=mybir.AluOpType.is_ge,
            fill=0.0, base=0, pattern=[[1, 128]], channel_multiplier=-1)

        for rep in range(repeat):
            r = f"r{rep}"
            xT = xtp.tile([128, DT, S], F32R, tag="xT", name=f"xT{r}")
            qkT = [qkp.tile([128, S], BF16, tag=f"qkT{fi}",
                            name=f"qkT{fi}{r}") for fi in range(2 * DT)]
            vpp = [vp.tile([128, H, DH + 1], BF16, tag=f"vpp{si}",
                           name=f"vpp{si}{r}") for si in range(ST)]
            oT = [otp.tile([128, S], BF16, tag=f"oT{fi}",
                           name=f"oT{fi}{r}") for fi in range(DT)]
            wv = wvp.tile([128, DT, S], F32R, tag="wv", name=f"wv{r}")
            wor = [wop.tile([128, D], BF16, tag=f"wor{fi}",
                            name=f"wor{fi}{r}") for fi in range(DT)]

            # ---- stage 0: load x chunks, transpose on PE, copy out on Act
            def emit_T(g, split=False):
                xs = xsp.tile([128, D], F32, tag="xs", name=f"xs{g}{r}")
                if split:
                    # halves in separate DMAs so the first transposes start
                    # ~0.7us earlier (startup is DMA-latency-bound)
                    nc.sync.dma_start(out=xs[:, 0:512], in_=x_v[:, g, 0:512])
                    nc.sync.dma_start(out=xs[:, 512:1024],
                                      in_=x_v[:, g, 512:1024])
                else:
                    nc.sync.dma_start(out=xs, in_=x_v[:, g, :])
                for half in range(2):
                    tp = psA.tile([128, 512], F32, tag="ps",
                                  name=f"tp{g}_{half}{r}")
                    for q in range(4):
                        di = half * 4 + q
                        nc.tensor.transpose(
                            tp[:, q * 128:(q + 1) * 128],
                            xs[:, di * 128:(di + 1) * 128], ident)
                    nc.scalar.activation(
                        xT[:, half * 4:(half + 1) * 4, g * 128:(g + 1) * 128],
                        tp.rearrange("p (q c) -> p q c", q=4), ACT_COPY)

            # ones columns for softmax denominators (Pool, SBUF-only)
            def emit_ones():
                for si in range(ST):
                    nc.gpsimd.memset(vpp[si][:, :, DH:DH + 1], 1.0)

            # ---- weight DMAs (queue order = need order) ----
            def load_wqr(fi):
                w = wqs.tile([128, DT, 128], F32R, tag="wqr",
                             name=f"wqr{fi}{r}")
                nc.sync.dma_start(out=w, in_=wqkv_v[:, :, fi * 128:(fi + 1) * 128])
                return w

            def gen_qkT(fi, w, cbs=(0, 1)):
                """Generator: yields pe_ns after every ~2 matmuls so attention
                emission can interleave (PE's in-order queue only skips 4
                blocked instructions, so long blocked PV runs stall it)."""
                for cb in cbs:
                    ps = psA.tile([128, 512], F32, tag="ps",
                                  name=f"qkacc{fi}_{cb}{r}")
                    for di in range(DT):
                        nc.tensor.matmul(
                            ps, w[:, di, :],
                            xT[:, di, cb * 512:(cb + 1) * 512],
                            start=(di == 0), stop=(di == DT - 1))
                        if di % 2 == 1:
                            yield 426
                    nc.vector.tensor_copy(
                        qkT[fi][:, cb * 512:(cb + 1) * 512], ps)
                    yield 0

            def emit_qkT(fi, w, cbs=(0, 1)):
                for _ in gen_qkT(fi, w, cbs):
                    pass

            def gen_v(vb, copy_engine):
                # v natural [s, f] for heads vb*8..vb*8+7 -> vpp bf16
                for si in range(ST):
                    ps = psA.tile([128, 512], F32, tag="ps",
                                  name=f"vacc{vb}_{si}{r}")
                    for di in range(DT):
                        nc.tensor.matmul(
                            ps, xT[:, di, si * 128:(si + 1) * 128],
                            wv[:, di, vb * 512:(vb + 1) * 512],
                            start=(di == 0), stop=(di == DT - 1))
                        if di % 2 == 1:
                            yield 426
                    copy_engine(
                        vpp[si][:, vb * 8:(vb + 1) * 8, 0:DH],
                        ps.rearrange("p (h d) -> p h d", h=8))
                    yield 0

            def emit_v(vb, copy_engine):
                for _ in gen_v(vb, copy_engine):
                    pass

            def act_copy(dst, src):
                nc.scalar.activation(dst, src, ACT_COPY)

            def dve_copy(dst, src):
                nc.vector.tensor_copy(dst, src)

            PIPE = 2

            # filler machinery: a FIFO of (label, generator) yielding pe_ns
            # chunks; attention emission drains fillers to cover its
            # Act-vs-PE deficit.  drain_until forces a dependency's filler
            # to be fully emitted before its consumer.
            fillers: list = []
            deficit = [0.0]

            def drain(ns):
                while ns > 0 and fillers:
                    try:
                        ns -= next(fillers[0][1])
                    except StopIteration:
                        fillers.pop(0)
                return ns

            def drain_until(label):
                while any(lb == label for lb, _ in fillers):
                    lb, g = fillers[0]
                    for _ in g:
                        pass
                    fillers.pop(0)

            def drain_all():
                while fillers:
                    drain(1e9)

            def emit_attn(qb, hp):
                drain_until(("qkT", hp))
                if hp >= 4:
                    drain_until(("v", 1))
                kmax = 4 * (qb + 1)
                fq, fk = hp, DT + hp
                o_ps = [psO.tile([DH + 1, 512], F32, tag="ops",
                                 name=f"o{qb}_{hp}_{j}{r}") for j in range(2)]
                pts: dict = {}

                def emit_scores(ki):
                    qs = max(0, ki * 128 - qb * 512)
                    n = 512 - qs
                    diag = ki * 128 >= qb * 512
                    for j in range(2):
                        p0 = j * 64
                        s_ps = psA.tile([128, 512], F32, tag="ps",
                                        name=f"s{qb}_{hp}_{ki}_{j}{r}")
                        nc.tensor.matmul(
                            s_ps[:, 0:n],
                            qkT[fk][p0:p0 + 64, ki * 128:(ki + 1) * 128],
                            qkT[fq][p0:p0 + 64, qb * 512 + qs:(qb + 1) * 512],
                            start=True, stop=True)
                        pt = ptp.tile([128, 512], BF16, tag="pt",
                                      name=f"pt{qb}_{hp}_{ki}_{j}{r}")
                        nc.scalar.activation(
                            pt[:, 0:n], s_ps[:, 0:n], ACT_EXP, scale=0.125)
                        if diag:
                            # causal: zero keys below the diagonal
                            nc.vector.tensor_mul(
                                pt[:, 0:128], pt[:, 0:128], trimask)
                        pts[(ki, j)] = (pt, qs, n)

                def emit_pv(ki):
                    for j in range(2):
                        pt, qs, n = pts.pop((ki, j))
                        nc.tensor.matmul(
                            o_ps[j][:, qs:512],
                            vpp[ki][:, 2 * hp + j, :], pt[:, 0:n],
                            start=(ki == 0), stop=(ki == kmax - 1))

                for ki in range(kmax):
                    emit_scores(ki)
                    pe = act = 0.0
                    n = 512 - max(0, ki * 128 - qb * 512)
                    pe += 2 * n * 0.4166
                    act += 2 * (n * 0.833 + 190)
                    if ki >= PIPE:
                        emit_pv(ki - PIPE)
                        pe += 2 * (512 - max(0, (ki - PIPE) * 128
                                             - qb * 512)) * 0.4166
                    deficit[0] += act - pe
                    if deficit[0] > 0:
                        deficit[0] = drain(deficit[0])
                for ki in range(max(0, kmax - PIPE), kmax):
                    emit_pv(ki)

                for j in range(2):
                    p0 = j * 64
                    # stage PSUM->SBUF bf16 right away: frees the psO bank
                    # ~0.6us after the last PV (the ring was stalling new
                    # units), and the rest of the epilogue runs in bf16
                    # SBUF where DVE gets its 2x mode
                    osb = obp.tile([DH + 1, 512], BF16, tag="osb",
                                   name=f"osb{qb}_{hp}_{j}{r}")
                    nc.vector.tensor_copy(osb, o_ps[j])
                    rs = rsp.tile([1, 512], BF16, tag="rs",
                                  name=f"rs{qb}_{hp}_{j}{r}")
                    with nc.allow_low_precision(
                            reason="bf16 softmax denominators; rel-err "
                                   "budget 2e-2, measured ~3e-3"):
                        nc.vector.reciprocal(rs, osb[DH:DH + 1, :])
                    bc = bcp.tile([64, 512], BF16, tag="bc",
                                  name=f"bc{qb}_{hp}_{j}{r}")
                    nc.gpsimd.partition_broadcast(bc, rs)
                    nc.vector.tensor_mul(
                        oT[hp][p0:p0 + 64, qb * 512:(qb + 1) * 512],
                        osb[0:DH, :], bc)

            def gen_out_proj(qb, copy_engine):
                for si in range(4 * qb, 4 * (qb + 1)):
                    for eb in range(2):
                        ps = psA.tile([128, 512], F32, tag="ps",
                                      name=f"op{si}_{eb}{r}")
                        for fi in range(DT):
                            nc.tensor.matmul(
                                ps, oT[fi][:, si * 128:(si + 1) * 128],
                                wor[fi][:, eb * 512:(eb + 1) * 512],
                                start=(fi == 0), stop=(fi == DT - 1))
                            if fi % 2 == 1:
                                yield 426
                        ostg = osp.tile([128, 512], F32, tag="ostg",
                                        name=f"ostg{si}_{eb}{r}")
                        copy_engine(ostg, ps)
                        nc.sync.dma_start(
                            out=out_d[si * 128:(si + 1) * 128,
                                      eb * 512:(eb + 1) * 512],
                            in_=ostg)
                        yield 0

            def out_proj(qb, copy_engine):
                for _ in gen_out_proj(qb, copy_engine):
                    pass

            # ---- schedule ----
            # Explicit interleave: attention units (qb,hp) spread across the
            # projection stream so Act's exp load (97us) never locally
            # outruns PE.  q1 units are Act-heavy (8.7us exp vs 3.7 for q0),
            # so they sit next to big PE segments.
            def load_wv(vb):
                for di in range(DT):
                    nc.sync.dma_start(
                        out=wv[:, di, vb * 512:(vb + 1) * 512],
                        in_=wqkv_d[di * 128:(di + 1) * 128,
                                   2 * D + vb * 512:2 * D + (vb + 1) * 512])

            def gen_wout(k):
                # wout via ordered sync queue (a dep-free gpsimd casting DMA
                # gets scheduled at t=0 and clogs startup DMA bandwidth);
                # f32->bf16 convert on DVE
                for fi in (2 * k, 2 * k + 1):
                    ws = wos.tile([128, D], F32R, tag="wos",
                                  name=f"wos{fi}{r}")
                    nc.sync.dma_start(
                        out=ws, in_=wout_d[fi * 128:(fi + 1) * 128, :])
                    nc.vector.tensor_copy(wor[fi], ws)
                    yield 0

            def filler_qkT(hp):
                # DMA now (queue position), matmuls drained later
                wq_ = load_wqr(hp)
                wk_ = load_wqr(hp + DT)
                fillers.append((("qkT", hp), gen_qkT(hp, wq_)))
                fillers.append((("qkT", hp), gen_qkT(hp + DT, wk_)))

            # startup: T(g0-3) -> qkT(h0) cb0 halves -> T(g4-7) -> rest,
            # DMA queue ordered to match (x0-3, wqr h0, x4-7, wv0, wqr h1,
            # wv1, then per-hp wqr / wout)
            for g in range(4):
                emit_T(g, split=(g == 0))
            w00 = load_wqr(0)
            w01 = load_wqr(DT)
            emit_qkT(0, w00, cbs=(0,))
            emit_qkT(DT, w01, cbs=(0,))
            for g in range(4, ST):
                emit_T(g)
            emit_ones()
            load_wv(0)
            emit_qkT(0, w00, cbs=(1,))
            emit_qkT(DT, w01, cbs=(1,))
            w10 = load_wqr(1)
            w11 = load_wqr(1 + DT)
            load_wv(1)
            emit_qkT(1, w10)
            emit_qkT(1 + DT, w11)
            emit_v(0, act_copy)
            # attention with projection fillers interleaved at matmul
            # granularity; filler order respects needs (qkT h+1 completes
            # during units of h-1)
            filler_qkT(2)
            fillers.append(("wout", gen_wout(0)))
            emit_attn(0, 0)
            emit_attn(1, 0)
            filler_qkT(3)
            fillers.append(("wout", gen_wout(1)))
            emit_attn(0, 1)
            emit_attn(1, 1)
            filler_qkT(4)
            fillers.append(("wout", gen_wout(2)))
            emit_attn(0, 2)
            emit_attn(1, 2)
            fillers.append((("v", 1), gen_v(1, dve_copy)))
            filler_qkT(5)
            fillers.append(("wout", gen_wout(3)))
            emit_attn(0, 3)
            emit_attn(1, 3)
            filler_qkT(6)
            emit_attn(0, 4)
            emit_attn(1, 4)
            filler_qkT(7)
            emit_attn(0, 5)
            emit_attn(1, 5)
            emit_attn(0, 6)
            emit_attn(1, 6)
            drain_all()
            emit_attn(0, 7)
            fillers.append(("oproj", gen_out_proj(0, dve_copy)))
            emit_attn(1, 7)
            drain_all()
            out_proj(1, act_copy)
    nc.compile()
    return nc


_nc_cache: dict = {}


def _get_nc(repeat: int = 1, cut: str = ""):
    if (repeat, cut) not in _nc_cache:
        _nc_cache[(repeat, cut)] = build_nc(repeat, cut)
    return _nc_cache[(repeat, cut)]


def run(x, W_qkv, W_out, repeat: int = 1):
    nc = _get_nc(repeat)
    x = np.ascontiguousarray(np.asarray(x, dtype=np.float32))
    W_qkv = np.ascontiguousarray(np.asarray(W_qkv, dtype=np.float32))
    W_out = np.ascontiguousarray(np.asarray(W_out, dtype=np.float32))
    in_maps = [{"x": x[b], "wqkv": W_qkv, "wout": W_out} for b in range(NCORES)]
    res = run_bass_kernel_spmd(nc, in_maps, core_ids=list(range(NCORES)))
    return np.stack([res.results[b]["out"] for b in range(NCORES)], axis=0)


def kernel(x, mask=None, W_qkv=None, W_out=None):
    """Full-input entry point; mask is always causal-tril and is hardcoded."""
    return run(x, W_qkv, W_out, repeat=1)
